# revision 38
# baseline (speedup 1.0000x reference)
"""Trainium2 Bass kernel: 4D-CNN ResNet Bottleneck block, SPMD over 8 NeuronCores.

Problem (hardcoded): x[2,256,8,16,16,16] ->
  relu(bn3(conv1x1_256(relu(bn2(conv3x3x3x3(relu(bn1(conv1x1_64(x)))))))) + x)
BatchNorms use training-mode batch stats over (B,T,D,H,W).

Sharding: 16 (b,t) slices -> 2 owned t-slices/core; each core's input slab
includes the +-1 t halo (zero padded at boundaries), so no activation
exchange is needed. Three tiny AllReduces merge the BN statistics.

conv2 (the dominant cost, 81-tap 4D conv 64->64) runs as 45 matmuls per
superplane instead of 81 by contracting TWO taps per matmul over K=128:
y1 is stored in a pitch-17 padded layout ((16+1)^3 per t-slice, pad slots
shared between neighboring windows) in TWO copies whose upper 64
partitions hold the same data shifted by +1 (kw+1) and +17 (kh+1)
elements respectively.  A K=128 matmul on copy A computes taps
(...,kh,kw=0)+(...,kh,kw=1); copy B pairs (..,kh=0,kw=2)+(..,kh=1,kw=2);
the 9 (kt,kd,kh=2,kw=2) leftovers run as K=64 singles.  27+9+9 = 45.
The shifted copies are built by 3 cheap SBUF->SBUF DMAs per t-slice from
the canonical lower half, which itself is written in place: conv1 stages
its raw output directly into the padded layout and BN1+relu is applied
in place after the stats AllReduce (masked per-slice to zero invalid
halo slices).

Precision: conv1/conv2 run fp16 (10-bit mantissa), conv3 runs float32r
(full-rate fp32 mode; operands must be written by a rounding ACT/DVE op).
The residual is injected into conv3's PSUM by an extra matmul against
diag(1/bn3_scale) with fp16 x, so the whole epilogue
relu(scale*psum + bias) is a single ScalarE pass.  BN accumulators and
all conv accumulation stay fp32.
"""

import functools

import numpy as np

# ---- problem constants --------------------------------------------------
B, C, T, D, H, W = 2, 256, 8, 16, 16, 16
PL = 64            # bottleneck planes
O3 = 4 * PL        # final channels (256)
NCORES = 8
EPS = 1e-5

TPC = 2                    # owned t-slices per core
SLAB = TPC + 2             # slab slices incl halo
DHW = D * H * W            # 4096
NSP_OWN = TPC * (D // 2)   # 16 owned superplanes (d-pairs)
NPOS_OWN = TPC * DHW       # 8192 positions per core

# pitch-17 padded y1 layout: slot 16 of each dim is the shared zero pad
P17 = 17
PLANE = P17 * P17          # 289
SLC = P17 * PLANE          # 4913 elems per padded t-slice
MARGIN = 320
SLABCOLS = MARGIN + SLAB * SLC + MARGIN   # 20292
Y1CCOLS = MARGIN + 2 * SLC + MARGIN       # 10466

MM_DT = "float16"    # conv3 matmul dtype
C2_DT = "float16"    # conv1/conv2 matmul dtype

LAST_RESULT = None  # BassKernelResults of the most recent run (for test.py)


def _bricks():
    """conv2 matmul plan: list of (delta, buf, k128).

    delta: rhs view offset relative to the superplane base col
    buf:   0 = copy A (upper half shifted +1 / kw+1),
           1 = copy B (upper half shifted +17 / kh+1)
    k128:  True -> contract 128 partitions (2 taps), False -> 64 (single)
    Weight block b holds lhsT columns [b*64, (b+1)*64).
    """
    out = []
    for kt in range(3):
        for kd in range(3):
            for kh in range(3):  # taps (kt,kd,kh,0) + (kt,kd,kh,1)
                out.append(((kt - 1) * SLC + (kd - 1) * PLANE
                            + (kh - 1) * P17 - 1, 0, True))
    for kt in range(3):
        for kd in range(3):      # taps (kt,kd,0,2) + (kt,kd,1,2)
            out.append(((kt - 1) * SLC + (kd - 1) * PLANE
                        - P17 + 1, 1, True))
    for kd in range(3):          # taps (0,kd,2,2) + (1,kd,2,2) via copy C
        out.append(((kd - 1) * PLANE + P17 + 1, 2, True))
    for kd in range(3):          # tap (2,kd,2,2) single
        out.append((SLC + (kd - 1) * PLANE + P17 + 1, 0, False))
    return out


BRICKS = _bricks()
NBRICK = len(BRICKS)  # 45


@functools.lru_cache(maxsize=4)
def _build(mm_dt_name, c2_dt_name, collectives=True):
    from contextlib import ExitStack

    import concourse.bass as bass
    import concourse.mybir as mybir
    import concourse.tile as tile
    from concourse import bacc

    f32 = mybir.dt.float32
    fp16 = mybir.dt.float16
    mmdt = getattr(mybir.dt, mm_dt_name)
    c2dt = getattr(mybir.dt, c2_dt_name)
    AF = mybir.ActivationFunctionType
    AL = mybir.AluOpType

    nc = bacc.Bacc(
        "TRN2",
        target_bir_lowering=False,
        debug=False,
        enable_asserts=False,
        num_devices=NCORES,
    )

    xsb = nc.dram_tensor("xsb", [2, 128, SLAB * DHW], c2dt,
                         kind="ExternalInput").ap()
    idm = nc.dram_tensor("idm", [128, 128], fp16, kind="ExternalInput").ap()
    w1t = nc.dram_tensor("w1t", [128, 2 * PL], c2dt, kind="ExternalInput").ap()
    w2t = nc.dram_tensor("w2t", [128, NBRICK * PL], c2dt,
                         kind="ExternalInput").ap()
    w3t = nc.dram_tensor("w3t", [128, O3], fp16, kind="ExternalInput").ap()
    gb1 = nc.dram_tensor("gb1", [64, 2], f32, kind="ExternalInput").ap()
    gb2 = nc.dram_tensor("gb2", [128, 2], f32, kind="ExternalInput").ap()
    gb3 = nc.dram_tensor("gb3", [128, 4], f32, kind="ExternalInput").ap()
    tmask = nc.dram_tensor("tmask", [64, SLAB], f32, kind="ExternalInput").ap()
    out = nc.dram_tensor("out", [2, 128, NPOS_OWN], f32,
                         kind="ExternalOutput").ap()

    cc1_in = nc.dram_tensor("cc1_in", [64, 2], f32).ap()
    cc1_out = nc.dram_tensor("cc1_out", [64, 2], f32, addr_space="Shared").ap()
    cc2_in = nc.dram_tensor("cc2_in", [128, 2], f32).ap()
    cc2_out = nc.dram_tensor("cc2_out", [128, 2], f32, addr_space="Shared").ap()
    # BN3 statistics travel as the Gram matrix C = y2n @ y2n.T plus the
    # per-channel position sums (column 64)
    cc3_in = nc.dram_tensor("cc3_in", [64, 65], f32).ap()
    cc3_out = nc.dram_tensor("cc3_out", [64, 65], f32, addr_space="Shared").ap()
    RG = [list(range(NCORES))]

    def allreduce(cin, cout):
        if collectives:
            nc.gpsimd.collective_compute(
                "AllReduce", AL.add, replica_groups=RG,
                ins=[cin], outs=[cout],
            )
        else:  # timing-sim variant: stand-in DMA with the same deps
            nc.sync.dma_start(out=cout, in_=cin)

    with tile.TileContext(nc) as tc, ExitStack() as st:
        const = st.enter_context(tc.tile_pool(name="const", bufs=1))
        smalls = st.enter_context(tc.tile_pool(name="smalls", bufs=1))

        def sm(shape, nm):
            return smalls.tile(shape, f32, tag=nm, name=nm)

        # ---- persistent SBUF tensors ---------------------------------
        y1a = const.tile([128, SLABCOLS], c2dt, tag="y1a", name="y1a")
        y1b = const.tile([128, SLABCOLS], c2dt, tag="y1b", name="y1b")
        # copy C: slices 0-1 only, upper 64 partitions hold the NEXT
        # t-slice (kt+1 pairing for the (kh=2,kw=2) leftovers)
        y1c = const.tile([128, Y1CCOLS], c2dt, tag="y1c", name="y1c")
        w1sb = const.tile([128, 2 * PL], c2dt, tag="w1sb", name="w1sb")
        w2sb = const.tile([128, NBRICK * PL], c2dt, tag="w2sb", name="w2sb")
        w3sb = const.tile([128, O3], mmdt, tag="w3sb", name="w3sb")
        idmsb = const.tile([128, 128], fp16, tag="idmsb", name="idmsb")
        diag3 = const.tile([128, 256], fp16, tag="diag3", name="diag3")

        gb1sb = smalls.tile([64, 2], f32, tag="gb1sb", name="gb1sb")
        gb2sb = sm([128, 2], "gb2sb")
        gb3sb = sm([128, 4], "gb3sb")
        tmsb = smalls.tile([64, SLAB], f32, tag="tmsb", name="tmsb")
        st1 = smalls.tile([64, NSP_OWN * 6], f32, tag="st1", name="st1")
        st2 = sm([128, 8 * 6], "st2")
        st3 = sm([128, 32 * 6], "st3")

        nc.sync.dma_start(out=w1sb[:], in_=w1t[:])
        nc.sync.dma_start(out=w3sb[:], in_=w3t[:])
        nc.sync.dma_start(out=gb1sb[:], in_=gb1[:])
        nc.sync.dma_start(out=tmsb[:], in_=tmask[:])
        nc.sync.dma_start(out=w2sb[:], in_=w2t[:])
        nc.sync.dma_start(out=idmsb[:], in_=idm[:])
        nc.sync.dma_start(out=gb2sb[:], in_=gb2[:])
        nc.sync.dma_start(out=gb3sb[:], in_=gb3[:])

        # zero y1a fully (pad slots + margins), owned slices first so
        # conv1's staging writes are unblocked early.  y1b only needs its
        # margins zeroed: the copy/stitch DMAs cover everything else.
        for s in (1, 2):
            half_ = SLC // 2
            a = MARGIN + s * SLC
            nc.gpsimd.memset(y1a[:, a:a + half_], 0.0)
            nc.gpsimd.memset(y1a[:, a + half_:a + SLC], 0.0)
        nc.gpsimd.memset(y1a[:, 0:MARGIN + SLC], 0.0)
        nc.gpsimd.memset(y1a[:, MARGIN + 3 * SLC:SLABCOLS], 0.0)
        nc.gpsimd.memset(y1b[:, 0:MARGIN], 0.0)
        nc.gpsimd.memset(y1b[:, MARGIN + SLAB * SLC - P17:SLABCOLS], 0.0)
        nc.gpsimd.memset(y1c[:, 0:MARGIN], 0.0)
        nc.gpsimd.memset(y1c[:, MARGIN + 2 * SLC:Y1CCOLS], 0.0)

        # ---- strided view helpers ------------------------------------
        def y1_view(buf, col, npart):
            """[npart, d(2), h(16), w(16)] window of the pitch-17 layout."""
            pitch = buf.ap[0][0]
            return bass.AP(buf.tensor, buf.offset + col,
                           [[pitch, npart], [PLANE, 2], [P17, 16], [1, 16]])

        def sp_base(s, dp):
            return MARGIN + s * SLC + 2 * dp * PLANE

        # ---- BN finalize helpers -------------------------------------
        def bn_reduce_prep(mv, arin):
            """arin[:,0]=local mean, arin[:,1]=local E[x^2]."""
            t = sm([mv.shape[0], 1], f"bnprep_t_{arin.name}")
            nc.vector.tensor_tensor(out=t[:], in0=mv[:, 0:1], in1=mv[:, 0:1],
                                    op=AL.mult)
            nc.vector.tensor_tensor(out=arin[:, 1:2], in0=mv[:, 1:2], in1=t[:],
                                    op=AL.add)
            nc.vector.tensor_copy(out=arin[:, 0:1], in_=mv[:, 0:1])

        def bn_finalize(sum0, sum1, inv_n, g_ap, b_ap, scale, bias, nm):
            """sum0=sum(mean_l), sum1=sum(e2_l) [P,1] -> scale/bias [P,1]."""
            P = scale.shape[0]
            mg = sm([P, 1], f"mg_{nm}")
            e2 = sm([P, 1], f"e2_{nm}")
            tt = sm([P, 1], f"tt_{nm}")
            nc.vector.tensor_scalar_mul(mg[:], sum0, inv_n)
            nc.vector.tensor_scalar_mul(e2[:], sum1, inv_n)
            nc.vector.tensor_tensor(out=tt[:], in0=mg[:], in1=mg[:], op=AL.mult)
            nc.vector.tensor_tensor(out=e2[:], in0=e2[:], in1=tt[:],
                                    op=AL.subtract)
            nc.vector.tensor_scalar_add(e2[:], e2[:], EPS)
            nc.vector.reciprocal(out=tt[:], in_=e2[:])
            rstd = sm([P, 1], f"rstd_{nm}")
            nc.scalar.activation(rstd[:], tt[:], AF.Sqrt)
            nc.vector.tensor_tensor(out=scale[:], in0=g_ap, in1=rstd[:],
                                    op=AL.mult)
            nc.vector.tensor_tensor(out=tt[:], in0=mg[:], in1=scale[:],
                                    op=AL.mult)
            nc.vector.tensor_tensor(out=bias[:], in0=b_ap, in1=tt[:],
                                    op=AL.subtract)

        scale1 = smalls.tile([64, 1], f32, tag="scale1", name="scale1")
        bias1 = smalls.tile([64, 1], f32, tag="bias1", name="bias1")
        scale2 = sm([128, 1], "scale2")
        bias2 = sm([128, 1], "bias2")
        scale3 = sm([128, 2], "scale3")
        bias3 = sm([128, 2], "bias3")

        # ======== conv1: 256->64, computed twice ======================
        # Pass 1 computes only the owned superplanes to get the BN1 stats
        # AllReduce in flight as early as possible; pass 2 recomputes all
        # 32 superplanes (the PE is idle here anyway) and applies
        # BN1+relu straight from PSUM into the padded y1a layout, masked
        # per-slice to zero invalid halo slices.  After each slice, DMAs
        # build the shifted upper halves / copy B.  Slice 3 (the back
        # halo) is deferred until after conv2 k=0..3, which do not read
        # it, so its matmuls and applies hide under early conv2.
        xho = st.enter_context(tc.tile_pool(name="xho", bufs=1))
        xslc = {}

        def stream_x(s, pl):
            for cb in range(2):
                tr = pl.tile([128, DHW], c2dt, tag=f"xs{cb}_{s}",
                             name=f"xs{cb}_{s}")
                nc.sync.dma_start(
                    out=tr[:], in_=xsb[cb, :, s * DHW:(s + 1) * DHW])
                xslc[(cb, s)] = tr

        def get_xt9(idx):
            sp, oh = divmod(idx, 2)
            s = 1 + sp // 8
            dp = sp % 8
            return xslc[(oh, s)][:, dp * 512:(dp + 1) * 512]

        mid = st.enter_context(tc.tile_pool(name="mid", bufs=1))
        y2 = mid.tile([128, NSP_OWN * 256], fp16, tag="y2", name="y2")
        y2n = mid.tile([128, NSP_OWN * 256], mmdt, tag="y2n", name="y2n")

        with tc.tile_pool(name="ps1", bufs=4, space="PSUM") as ps1, \
             tc.tile_pool(name="xh3", bufs=1) as xh3:

            def c1_mm(s, dp):
                ps = ps1.tile([64, 512], f32, tag="c1p", name="c1p")
                for cb in range(2):
                    nc.tensor.matmul(
                        ps[:],
                        lhsT=w1sb[:, cb * PL:(cb + 1) * PL],
                        rhs=xslc[(cb, s)][:, dp * 512:(dp + 1) * 512],
                        start=(cb == 0), stop=(cb == 1),
                    )
                return ps

            def slice_pass2(s, sc_s, bi_s):
                for dp in range(D // 2):
                    ps = c1_mm(s, dp)
                    v = y1_view(y1a, sp_base(s, dp), 64)
                    if dp % 4 == 3:
                        nc.vector.tensor_scalar(
                            v, ps[:], sc_s[s][:], bi_s[s][:],
                            op0=AL.mult, op1=AL.add)
                        nc.vector.tensor_scalar_max(v, v, 0.0)
                    else:
                        nc.scalar.activation(
                            v, ps[:], AF.Relu,
                            bias=bi_s[s][:], scale=sc_s[s][:])
                    if dp == 0:
                        # stitch the shifted-copy tails of the preceding
                        # region (slice s-1 or the front margin), which
                        # read this slice's first rows
                        ae = MARGIN + s * SLC
                        nc.sync.dma_start(
                            out=y1a[64:128, ae - 1:ae],
                            in_=y1a[0:64, ae:ae + 1])
                        nc.sync.dma_start(
                            out=y1b[64:128, ae - P17:ae],
                            in_=y1a[0:64, ae:ae + P17])
                a = MARGIN + s * SLC
                nc.sync.dma_start(out=y1a[64:128, a:a + SLC - 1],
                                  in_=y1a[0:64, a + 1:a + SLC])
                nc.vector.tensor_copy(out=y1b[0:64, a:a + SLC],
                                      in_=y1a[0:64, a:a + SLC])
                nc.sync.dma_start(out=y1b[64:128, a:a + SLC - P17],
                                  in_=y1a[0:64, a + P17:a + SLC])

            with tc.tile_pool(name="xh0", bufs=1) as xh0:
                stream_x(1, xho)
                stream_x(2, xho)
                stream_x(0, xh0)
                stream_x(3, xh3)

                for s in (1, 2):
                    for dp in range(D // 2):
                        ps = c1_mm(s, dp)
                        sp = (s - 1) * 8 + dp
                        nc.vector.bn_stats(out=st1[:, sp * 6:(sp + 1) * 6],
                                           in_=ps[:])

                mv1 = smalls.tile([64, 2], f32, tag="mv1", name="mv1")
                arin1 = smalls.tile([64, 2], f32, tag="arin1", name="arin1")
                nc.vector.bn_aggr(out=mv1[:], in_=st1[:])
                bn_reduce_prep(mv1, arin1)
                nc.sync.dma_start(out=cc1_in[:], in_=arin1[:])
                allreduce(cc1_in[:], cc1_out[:])
                g1s = smalls.tile([64, 2], f32, tag="g1s", name="g1s")
                nc.sync.dma_start(out=g1s[:], in_=cc1_out[:])
                bn_finalize(g1s[:, 0:1], g1s[:, 1:2], 1.0 / NCORES,
                            gb1sb[:, 0:1], gb1sb[:, 1:2],
                            scale1, bias1, "bn1")
                # per-slab-slice masked scale/bias (zero invalid slices)
                sc_s, bi_s = [], []
                for s in range(SLAB):
                    scs = smalls.tile([64, 1], f32, tag=f"sc1_{s}",
                                      name=f"sc1_{s}")
                    bis = smalls.tile([64, 1], f32, tag=f"bi1_{s}",
                                      name=f"bi1_{s}")
                    nc.vector.tensor_tensor(out=scs[:], in0=scale1[:],
                                            in1=tmsb[:, s:s + 1], op=AL.mult)
                    nc.vector.tensor_tensor(out=bis[:], in0=bias1[:],
                                            in1=tmsb[:, s:s + 1], op=AL.mult)
                    sc_s.append(scs)
                    bi_s.append(bis)

                # dp-major order with the shift DMAs split at the
                # d9/d10 boundary: conv2 k=0..1 (which only read
                # d-planes <= 9 of slices 0-2) unblock after the dp<=4
                # rounds instead of after the whole apply burst.
                HB = 10 * PLANE
                for dp in range(D // 2):
                    for s in (0, 1, 2):
                        ps = c1_mm(s, dp)
                        v = y1_view(y1a, sp_base(s, dp), 64)
                        if dp % 4 == 3:
                            nc.vector.tensor_scalar(
                                v, ps[:], sc_s[s][:], bi_s[s][:],
                                op0=AL.mult, op1=AL.add)
                            nc.vector.tensor_scalar_max(v, v, 0.0)
                        else:
                            nc.scalar.activation(
                                v, ps[:], AF.Relu,
                                bias=bi_s[s][:], scale=sc_s[s][:])
                        if dp == 0:
                            ae = MARGIN + s * SLC
                            nc.sync.dma_start(
                                out=y1a[64:128, ae - 1:ae],
                                in_=y1a[0:64, ae:ae + 1])
                            nc.sync.dma_start(
                                out=y1b[64:128, ae - P17:ae],
                                in_=y1a[0:64, ae:ae + P17])
                    if dp == 4:
                        for s in (0, 1, 2):
                            a = MARGIN + s * SLC
                            nc.sync.dma_start(
                                out=y1a[64:128, a:a + HB - 1],
                                in_=y1a[0:64, a + 1:a + HB])
                            nc.vector.tensor_copy(
                                out=y1b[0:64, a:a + HB],
                                in_=y1a[0:64, a:a + HB])
                            nc.sync.dma_start(
                                out=y1b[64:128, a:a + HB - P17],
                                in_=y1a[0:64, a + P17:a + HB])
                for s in (0, 1, 2):
                    a = MARGIN + s * SLC
                    nc.sync.dma_start(
                        out=y1a[64:128, a + HB - 1:a + SLC - 1],
                        in_=y1a[0:64, a + HB:a + SLC])
                    nc.vector.tensor_copy(
                        out=y1b[0:64, a + HB:a + SLC],
                        in_=y1a[0:64, a + HB:a + SLC])
                    nc.sync.dma_start(
                        out=y1b[64:128, a + HB - P17:a + SLC - P17],
                        in_=y1a[0:64, a + HB:a + SLC])
                # copy C: aligned whole-slice copies, consumed only by the
                # tail bricks of each conv2 chain (so these stream during
                # early conv2 while the DMA engines are otherwise idle)
                for s in range(2):
                    a = MARGIN + s * SLC
                    nc.sync.dma_start(out=y1c[0:64, a:a + SLC],
                                      in_=y1a[0:64, a:a + SLC])
                    nc.sync.dma_start(out=y1c[64:128, a:a + SLC],
                                      in_=y1a[0:64, a + SLC:a + 2 * SLC])

            # ======== conv2: 81-tap 4D conv, 64->64, 42 matmuls/sp =====
            with tc.tile_pool(name="ps3", bufs=4, space="PSUM") as ps3:

                def conv2_k(k):
                    s = 1 + k // 4
                    dpP = (k % 4) * 2
                    ps = ps3.tile([128, 512], f32, tag="c2", name="c2")
                    for b, (delta, bufsel, k128) in enumerate(BRICKS):
                        lhsT = (w2sb[:, b * PL:(b + 1) * PL] if k128
                                else w2sb[0:64, b * PL:(b + 1) * PL])
                        buf = (y1a, y1b, y1c)[bufsel]
                        npart = 128 if k128 else 64
                        for half, dp in ((0, dpP), (1, dpP + 1)):
                            base = (sp_base(s - 1, dp) if bufsel == 2
                                    else sp_base(s, dp))
                            nc.tensor.matmul(
                                ps[half * 64:(half + 1) * 64, :],
                                lhsT=lhsT,
                                rhs=y1_view(buf, base + delta, npart),
                                start=(b == 0), stop=(b == NBRICK - 1))
                    nc.scalar.copy(out=y2[:, k * 512:(k + 1) * 512],
                                   in_=ps[:])
                    nc.vector.bn_stats(out=st2[:, k * 6:(k + 1) * 6],
                                       in_=y2[:, k * 512:(k + 1) * 512])

                for k in range(4):
                    conv2_k(k)
                slice_pass2(3, sc_s, bi_s)
                for k in range(4, 8):
                    conv2_k(k)

        # ======== BN2 stats merge (rows 0-63 and 64-127 are disjoint
        # position groups of the same 64 channels) =====================
        mv2 = sm([128, 2], "mv2")
        arin2 = sm([128, 2], "arin2")
        nc.vector.bn_aggr(out=mv2[:], in_=st2[:])
        bn_reduce_prep(mv2, arin2)
        nc.sync.dma_start(out=cc2_in[:], in_=arin2[:])
        allreduce(cc2_in[:], cc2_out[:])
        fa = sm([128, 2], "fa2")
        fb = sm([128, 2], "fb2")
        nc.sync.dma_start(out=fa[:], in_=cc2_out[:])
        nc.sync.dma_start(out=fb[0:64, :], in_=cc2_out[64:128, :])
        nc.sync.dma_start(out=fb[64:128, :], in_=cc2_out[0:64, :])
        nc.vector.tensor_tensor(out=fa[:], in0=fa[:], in1=fb[:], op=AL.add)
        bn_finalize(fa[:, 0:1], fa[:, 1:2], 1.0 / (2 * NCORES),
                    gb2sb[:, 0:1], gb2sb[:, 1:2], scale2, bias2, "bn2")

        for k in range(8):  # BN2 + relu (rounds to conv3 matmul dtype)
            dst = y2n[:, k * 512:(k + 1) * 512]
            srcb = y2[:, k * 512:(k + 1) * 512]
            if k % 2 == 1:
                nc.vector.tensor_scalar(dst, srcb, scale2[:], bias2[:],
                                        op0=AL.mult, op1=AL.add)
                nc.vector.tensor_scalar_max(dst, dst, 0.0)
            else:
                nc.scalar.activation(dst, srcb, AF.Relu,
                                     bias=bias2[:], scale=scale2[:])

        def c3_mm(ps4, k, half, oh, stop=True):
            rhs = y2n[half * 64:(half + 1) * 64,
                      k * 512:(k + 1) * 512].bitcast(mmdt)
            pg = ps4.tile([128, 512], f32, tag="c3")
            nc.tensor.matmul(
                pg[:],
                lhsT=w3sb[half * 64:(half + 1) * 64,
                          oh * 128:(oh + 1) * 128].bitcast(mmdt),
                rhs=rhs, start=True, stop=stop)
            return pg

        # ======== conv3 stats via the Gram matrix =================
        # E[y3_o^2] = w3[o]^T C w3[o] / N and mean_o = w3[o].m / N with
        # C = y2n y2n^T and m = row sums of y2n: no conv3 stats pass and
        # no DVE bn_stats burst.  y2n is transposed in 128-position
        # chunks on the PE, staged to SBUF in fp16 with a constant-one
        # 65th column, and C accumulates in one PSUM tile.
        stp = [const.tile([128, 65], fp16, tag=f"stp{i}", name=f"stp{i}")
               for i in range(8)]
        for i in range(8):
            nc.gpsimd.memset(stp[i][:, 64:65], 1.0)
        idr128 = const.tile([128, 64], mmdt, tag="idr128", name="idr128")
        nc.vector.tensor_copy(out=idr128[0:64, :], in_=idmsb[0:64, 0:64])
        nc.vector.tensor_copy(out=idr128[64:128, :], in_=idmsb[64:128, 64:128])
        with tc.tile_pool(name="ps4", bufs=6, space="PSUM") as ps4, \
             tc.tile_pool(name="psc", bufs=1, space="PSUM") as pscp:
            psC = [pscp.tile([64, 65], f32, tag=f"psC{i}", name=f"psC{i}")
                   for i in range(2)]
            nchunk = 2 * NSP_OWN * 256 // 128   # 64
            for j in range(nchunk):
                half, jc = divmod(j, nchunk // 2)
                chunk = y2n[half * 64:(half + 1) * 64,
                            jc * 128:(jc + 1) * 128].bitcast(mmdt)
                pt = ps4.tile([128, 64], mmdt, tag="c3t")
                nc.tensor.matmul(
                    pt[:], lhsT=chunk,
                    rhs=idr128[half * 64:(half + 1) * 64, :],
                    is_transpose=True, tile_position=(half * 64, 0))
                sti = stp[j % 8]
                if j % 2 == 0:
                    nc.vector.tensor_copy(out=sti[:, 0:64], in_=pt[:])
                else:
                    nc.scalar.copy(out=sti[:, 0:64], in_=pt[:])
                nc.tensor.matmul(psC[j % 2][:], lhsT=sti[:, 0:64], rhs=sti[:],
                                 start=(j < 2), stop=(j >= nchunk - 2))
            Csb = smalls.tile([64, 65], f32, tag="Csb", name="Csb")
            nc.vector.tensor_copy(out=Csb[:], in_=psC[0][:])
            nc.vector.tensor_tensor(out=Csb[:], in0=Csb[:], in1=psC[1][:],
                                    op=AL.add)
        nc.sync.dma_start(out=cc3_in[:], in_=Csb[:])
        allreduce(cc3_in[:], cc3_out[:])
        Cg = smalls.tile([64, 65], f32, tag="Cg", name="Cg")
        nc.sync.dma_start(out=Cg[:], in_=cc3_out[:])
        Cgr = const.tile([64, 65], mmdt, tag="Cgr", name="Cgr")
        nc.vector.tensor_copy(out=Cgr[:], in_=Cg[:])

        recip3 = sm([128, 2], "recip3")
        # mbr[c, :] = m[c] broadcast: lets diag(mbr^T w3) give the means
        mbr = const.tile([64, 128], mmdt, tag="mbr", name="mbr")
        nc.vector.tensor_copy(out=mbr[:],
                              in_=Cg[:, 64:65].to_broadcast([64, 128]))
        scr3 = sm([128, 128], "scr3")
        with tc.tile_pool(name="ps4b", bufs=1, space="PSUM") as ps4b:
            for oh in range(2):
                w3b = w3sb[0:64, oh * 128:(oh + 1) * 128].bitcast(mmdt)
                psV = ps4b.tile([64, 128], f32, tag="psV")
                nc.tensor.matmul(psV[:], lhsT=Cgr[:, 0:64].bitcast(mmdt),
                                 rhs=w3b, start=True, stop=True)
                Vsb = const.tile([64, 128], mmdt, tag=f"Vsb{oh}",
                                 name=f"Vsb{oh}")
                nc.vector.tensor_copy(out=Vsb[:], in_=psV[:])
                # diag(Vsb^T w3) = E[y3^2] sums; diag(mbr^T w3) = mean sums
                psF = ps4b.tile([128, 128], f32, tag="psF")
                nc.tensor.matmul(psF[:], lhsT=Vsb[:], rhs=w3b,
                                 start=True, stop=True)
                e2col = sm([128, 1], f"e2col{oh}")
                nc.vector.tensor_tensor(out=scr3[:], in0=psF[:],
                                        in1=idmsb[:], op=AL.mult)
                nc.vector.tensor_reduce(out=e2col[:], in_=scr3[:],
                                        axis=mybir.AxisListType.X, op=AL.add)
                psG = ps4b.tile([128, 128], f32, tag="psG")
                nc.tensor.matmul(psG[:], lhsT=mbr[:], rhs=w3b,
                                 start=True, stop=True)
                mcol = sm([128, 1], f"mcol{oh}")
                nc.vector.tensor_tensor(out=scr3[:], in0=psG[:],
                                        in1=idmsb[:], op=AL.mult)
                nc.vector.tensor_reduce(out=mcol[:], in_=scr3[:],
                                        axis=mybir.AxisListType.X, op=AL.add)
                bn_finalize(mcol[:], e2col[:],
                            1.0 / (NPOS_OWN * NCORES),
                            gb3sb[:, oh:oh + 1], gb3sb[:, 2 + oh:3 + oh],
                            scale3[:, oh:oh + 1], bias3[:, oh:oh + 1],
                            f"bn3_{oh}")
        nc.vector.reciprocal(out=recip3[:], in_=scale3[:])
        for oh in range(2):
            # diag(1/scale3): lets the PE inject the residual into PSUM
            nc.vector.tensor_scalar_mul(
                diag3[:, oh * 128:(oh + 1) * 128], idmsb[:],
                recip3[:, oh:oh + 1])

        # ==== conv3 pass 2 + fused BN3/residual/relu/store ========
        with tc.tile_pool(name="ps5", bufs=8, space="PSUM") as ps5, \
             tc.tile_pool(name="fino", bufs=4) as fino:
            for k in range(8):
                P, Q = 2 * k, 2 * k + 1
                o9 = [fino.tile([128, 1024], f32, tag="o9", name=f"o9_{k}_{i}")
                      for i in range(2)]
                for half, sp in ((0, P), (1, Q)):
                    for oh in range(2):
                        idx = sp * 2 + oh
                        pg = c3_mm(ps5, k, half, oh, stop=False)
                        nc.tensor.matmul(
                            pg[:], lhsT=diag3[:, oh * 128:(oh + 1) * 128],
                            rhs=get_xt9(idx)[:], start=False, stop=True)
                        dsl = o9[oh][:, half * 512:(half + 1) * 512]
                        if k % 4 == 1:
                            nc.vector.tensor_scalar(
                                dsl, pg[:], scale3[:, oh:oh + 1],
                                bias3[:, oh:oh + 1],
                                op0=AL.mult, op1=AL.add)
                            nc.vector.tensor_scalar_max(dsl, dsl, 0.0)
                        else:
                            nc.scalar.activation(
                                dsl, pg[:], AF.Relu,
                                bias=bias3[:, oh:oh + 1],
                                scale=scale3[:, oh:oh + 1])
                for oh in range(2):
                    nc.sync.dma_start(
                        out=out[oh, :, P * 512:(P + 2) * 512],
                        in_=o9[oh][:])

    nc.compile()
    return nc


# ---- host-side input prep / output assembly -----------------------------

def _prep_inputs(x, w1, g1, b1, w2, g2, b2, w3, g3, b3):
    f4 = np.float32
    h16 = np.float16
    xr = np.ascontiguousarray(x, f4).reshape(B, C, T, DHW)

    w2r = np.ascontiguousarray(w2, f4).reshape(PL, PL, 3, 3, 3, 3)
    blocks = []
    for kt in range(3):
        for kd in range(3):
            for kh in range(3):
                blocks.append((w2r[:, :, kt, kd, kh, 0].T,
                               w2r[:, :, kt, kd, kh, 1].T))
    for kt in range(3):
        for kd in range(3):
            blocks.append((w2r[:, :, kt, kd, 0, 2].T,
                           w2r[:, :, kt, kd, 1, 2].T))
    zz = np.zeros((PL, PL), f4)
    for kd in range(3):
        blocks.append((w2r[:, :, 0, kd, 2, 2].T, w2r[:, :, 1, kd, 2, 2].T))
    for kd in range(3):
        blocks.append((w2r[:, :, 2, kd, 2, 2].T, zz))
    w2t = np.concatenate(
        [np.concatenate([lo for lo, _ in blocks], 1),
         np.concatenate([up for _, up in blocks], 1)], 0).astype(h16)

    w1T = np.ascontiguousarray(w1, f4).T          # [256, 64]
    w1t = np.concatenate([w1T[0:128], w1T[128:256]], 1).astype(h16)
    w3t = np.concatenate([np.ascontiguousarray(w3, f4).T] * 2, 0).astype(h16)

    gb1 = np.stack([np.asarray(g1, f4), np.asarray(b1, f4)], 1)
    gb2 = np.stack([np.asarray(g2, f4), np.asarray(b2, f4)], 1)
    gb2 = np.concatenate([gb2, gb2], 0)
    g3r = np.asarray(g3, f4).reshape(2, 128).T
    b3r = np.asarray(b3, f4).reshape(2, 128).T
    gb3 = np.concatenate([g3r, b3r], 1).copy()  # [128,4]

    in_maps = []
    for core in range(NCORES):
        b = core // 4
        t0 = 2 * (core % 4)
        xslab = np.zeros((C, SLAB, DHW), f4)
        tm = np.zeros((SLAB,), f4)
        for si, gt in enumerate(range(t0 - 1, t0 + 3)):
            if 0 <= gt < T:
                xslab[:, si] = xr[b, :, gt]
                tm[si] = 1.0
        xs2 = xslab.reshape(2, 128, SLAB * DHW)
        in_maps.append({
            "xsb": np.ascontiguousarray(xs2).astype(h16),
            "idm": np.eye(128, dtype=h16),
            "w1t": w1t, "w2t": w2t, "w3t": w3t,
            "gb1": gb1, "gb2": gb2, "gb3": gb3,
            "tmask": np.broadcast_to(tm, (64, SLAB)).copy(),
        })
    return in_maps


def kernel(x, w1, g1, b1, w2, g2, b2, w3, g3, b3):
    global LAST_RESULT
    from concourse.bass_utils import run_bass_kernel_spmd

    nc = _build(MM_DT, C2_DT)
    in_maps = _prep_inputs(x, w1, g1, b1, w2, g2, b2, w3, g3, b3)
    res = run_bass_kernel_spmd(nc, in_maps, core_ids=list(range(NCORES)))
    LAST_RESULT = res

    full = np.empty((B, C, T, D, H, W), np.float32)
    for core in range(NCORES):
        b = core // 4
        t0 = 2 * (core % 4)
        o = res.results[core]["out"].reshape(C, TPC, D, H, W)
        full[b, :, t0:t0 + TPC] = o
    return full
